# revision 2
# baseline (speedup 1.0000x reference)
"""MoE expert-MLP kernel for Trainium2, expert-parallel across 8 NeuronCores.

v2: single pass over the full capacity C (no C-halving), all-bf16 operands,
minimal HBM traffic. PE microbenchmarks show MM spacing = 216ns (bf16,
N=512) when clean, plus ~0.36ns per GB/s of concurrent DMA traffic (SBUF
port contention). So W1/W2 are streamed exactly once in bf16, xt/sc/yt are
bf16, totaling ~34MB per core vs 101MB for the f32r two-pass variant.

Problem: T=8192 tokens, H=1024 hidden, I=4096 intermediate, E=8 experts,
top-K=2, capacity C = T*K/E = 2048 slots per expert. One expert per core.

Device program per core:
  phase 1: for each of 32 m-tile pairs of W1 (gate|up packed [128,2048]):
    gu^T = W1^T x^T accumulated over H in PSUM (pg, pu), evacuated as
    h^T = silu(g^T)*u^T into a resident bf16 ht buffer [I=4096, C].
  phase 2: y^T = W2^T h^T accumulated over I, scaled by the per-slot
    combine scale on evacuation, written out in bf16.

xt DRAM/SBUF layout is chunk-major [128, 4 chunks x (8 k-tiles x 512)] so
each 512-column chunk loads as one contiguous 1MB DMA.
"""

import sys

import numpy as np

try:
    import concourse.bass as bass  # noqa: F401
except ImportError:
    for p in ("/opt/trn_rl_repo", "/root/.axon_site/_ro/trn_rl_repo"):
        if p not in sys.path:
            sys.path.insert(0, p)

import ml_dtypes
import concourse.bass as bass
import concourse.tile as tile
from concourse import bacc, mybir
from concourse.bass_utils import run_bass_kernel_spmd

dt = mybir.dt

T, H, I, E, K = 8192, 1024, 4096, 8, 2
C = 2048  # min(T, ceil(T*K*1.0/E))
KH = H // 128    # 8  k-tiles for the first contraction
KI = I // 128    # 32 k-tiles for the second contraction
MH = I // 128    # 32 m-tile pairs (gate, up) in phase 1
M2 = H // 128    # 8  output tiles of y^T
NCH = C // 512   # 4  512-column chunks

_CACHED_NC = None


def _build_nc():
    nc = bacc.Bacc(None)
    xt = nc.dram_tensor("xt", [128, KH * C], dt.bfloat16, kind="ExternalInput")
    w1 = nc.dram_tensor("w1", [MH, 128, 2048], dt.bfloat16, kind="ExternalInput")
    w2 = nc.dram_tensor("w2", [M2, 128, KI * 128], dt.bfloat16, kind="ExternalInput")
    sc = nc.dram_tensor("sc", [128, C], dt.bfloat16, kind="ExternalInput")
    yt = nc.dram_tensor("yt", [M2, 128, C], dt.bfloat16, kind="ExternalOutput")

    with tile.TileContext(nc) as tc:
        with (
            tc.tile_pool(name="xt_p", bufs=1) as xt_p,
            tc.tile_pool(name="sc_p", bufs=1) as sc_p,
            tc.tile_pool(name="ht_p", bufs=1) as ht_p,
            tc.tile_pool(name="w1_p", bufs=2) as w1_p,
            tc.tile_pool(name="w2_p", bufs=2) as w2_p,
            tc.tile_pool(name="sg_p", bufs=4) as sg_p,
            tc.tile_pool(name="yb_p", bufs=2) as yb_p,
            tc.tile_pool(name="ps", bufs=2, space="PSUM") as ps,
            tc.tile_pool(name="ps_y", bufs=4, space="PSUM") as ps_y,
        ):
            xt_t = xt_p.tile([128, KH * C], dt.bfloat16, tag="xt", name="xt")
            sc_t = sc_p.tile([128, C], dt.bfloat16, tag="sc", name="sc")
            ht_t = [ht_p.tile([128, C], dt.bfloat16, tag=f"ht{i}", name=f"ht{i}")
                    for i in range(KI)]

            # Startup: xt chunk 0 gates the first MM group — split it
            # across both HWDGE issue engines (sync + scalar) so the two
            # halves issue concurrently; w1[0] likewise.
            w1t0 = w1_p.tile([128, 2048], dt.bfloat16, tag="w1", name="w1t0")
            half = KH * 256
            nc.sync.dma_start(xt_t[:, 0:half], xt[:, 0:half])
            nc.scalar.dma_start(xt_t[:, half:2 * half], xt[:, half:2 * half])
            nc.sync.dma_start(w1t0[:, 0:1024], w1[0][:, 0:1024])
            nc.scalar.dma_start(w1t0[:, 1024:2048], w1[0][:, 1024:2048])
            w1t1 = w1_p.tile([128, 2048], dt.bfloat16, tag="w1", name="w1t1")
            nc.scalar.dma_start(w1t1[:], w1[1])
            nc.sync.dma_start(
                xt_t[:, KH * 512:2 * KH * 512], xt[:, KH * 512:2 * KH * 512])
            for q in range(2, NCH):
                nc.sync.dma_start(
                    xt_t[:, q * KH * 512:(q + 1) * KH * 512],
                    xt[:, q * KH * 512:(q + 1) * KH * 512],
                )
            nc.sync.dma_start(sc_t[:], sc[:])

            # ---- phase 1: gu^T = W1^T x^T ; h^T = silu(g)*u (bf16) ----
            for mh in range(MH):
                if mh == 0:
                    w1t = w1t0
                elif mh == 1:
                    w1t = w1t1
                else:
                    w1t = w1_p.tile([128, 2048], dt.bfloat16, tag="w1",
                                    name="w1t")
                    nc.sync.dma_start(w1t[:], w1[mh])
                if mh == 16 or mh == 24:
                    # prefetch W2 for the phase transition
                    w2t = w2_p.tile([128, KI * 128], dt.bfloat16, tag="w2",
                                    name="w2pre")
                    nc.sync.dma_start(w2t[:], w2[0 if mh == 16 else 1])
                    if mh == 16:
                        w2_pre = [w2t]
                    else:
                        w2_pre.append(w2t)
                ht = ht_t[mh]
                for n in range(NCH):
                    base = n * KH * 512
                    pg = ps.tile([128, 512], dt.float32, tag="pg", name="pg")
                    pu = ps.tile([128, 512], dt.float32, tag="pu", name="pu")
                    for k in range(KH):
                        nc.tensor.matmul(
                            pg[:],
                            w1t[:, k * 128:(k + 1) * 128],
                            xt_t[:, base + k * 512:base + (k + 1) * 512],
                            start=(k == 0),
                            stop=(k == KH - 1),
                        )
                    for k in range(KH):
                        nc.tensor.matmul(
                            pu[:],
                            w1t[:, 1024 + k * 128:1024 + (k + 1) * 128],
                            xt_t[:, base + k * 512:base + (k + 1) * 512],
                            start=(k == 0),
                            stop=(k == KH - 1),
                        )
                    sg = sg_p.tile([128, 512], dt.float32, tag="sg", name="sg")
                    nc.scalar.activation(
                        sg[:], pg[:], mybir.ActivationFunctionType.Silu
                    )
                    nc.vector.tensor_mul(
                        ht[:, n * 512:(n + 1) * 512], sg[:], pu[:]
                    )

            # ---- phase 2: y^T = W2^T h^T, scaled on evacuation ----
            for m in range(M2):
                if m < 2:
                    w2t = w2_pre[m]
                else:
                    w2t = w2_p.tile([128, KI * 128], dt.bfloat16, tag="w2",
                                    name="w2t")
                    nc.sync.dma_start(w2t[:], w2[m])
                yb = yb_p.tile([128, C], dt.bfloat16, tag="yb", name="yb")
                for n in range(NCH):
                    py = ps_y.tile([128, 512], dt.float32, tag="py", name="py")
                    for k in range(KI):
                        nc.tensor.matmul(
                            py[:],
                            w2t[:, k * 128:(k + 1) * 128],
                            ht_t[k][:, n * 512:(n + 1) * 512],
                            start=(k == 0),
                            stop=(k == KI - 1),
                        )
                    nc.vector.tensor_mul(yb[:, n * 512:(n + 1) * 512], py[:],
                                         sc_t[:, n * 512:(n + 1) * 512])
                    if m == M2 - 1:
                        nc.sync.dma_start(yt[m][:, n * 512:(n + 1) * 512],
                                          yb[:, n * 512:(n + 1) * 512])
                if m < M2 - 1:
                    nc.sync.dma_start(yt[m], yb[:])
    nc.finalize()
    return nc


def _route(expert_affinities, expert_index):
    """Numpy port of the reference routing. Returns (tok_idx, valid, scale)."""
    mask = np.zeros((T, E), dtype=np.float32)
    rows = np.arange(T)[:, None]
    mask[rows, expert_index] = 1.0  # top-k entries are distinct per token
    position = np.cumsum(mask, axis=0, dtype=np.float32)  # 1-based
    mask = np.where(position > C, 0.0, mask)

    affin = np.where(mask == 0, 0.0, expert_affinities)
    denom = np.maximum(np.sum(np.abs(affin), axis=1, keepdims=True), 1e-12)
    affin = affin / denom

    offsets = np.arange(E, dtype=np.float32) * C
    pos_off = np.where(mask == 0, 0.0, position + offsets)
    perm_idx = np.take_along_axis(pos_off, expert_index, axis=1).astype(np.int32)

    tok_ids = np.broadcast_to(
        np.arange(1, T + 1, dtype=np.int32)[:, None], (T, K)
    )
    assignments = np.zeros(E * C + 1, dtype=np.int32)
    assignments[perm_idx.reshape(-1)] = tok_ids.reshape(-1)
    assignments = assignments[1:].reshape(E, C) - 1
    valid = assignments >= 0
    tok_idx = np.maximum(assignments, 0)

    scale = affin[tok_idx, np.arange(E)[:, None]] * valid.astype(np.float32)
    return tok_idx, valid, scale, perm_idx


def prepare_in_maps(hidden_states, expert_affinities, expert_index,
                    gate_up_proj, down_proj):
    hidden_states = np.asarray(hidden_states, dtype=np.float32)
    expert_affinities = np.asarray(expert_affinities, dtype=np.float32)
    expert_index = np.asarray(expert_index, dtype=np.int32)
    gate_up_proj = np.asarray(gate_up_proj, dtype=np.float32)
    down_proj = np.asarray(down_proj, dtype=np.float32)

    tok_idx, valid, scale, perm_idx = _route(expert_affinities, expert_index)

    def _prep_expert(e):
        x_e = hidden_states[tok_idx[e]]  # (C, H)
        xT = np.ascontiguousarray(x_e.T)  # (H, C)
        # chunk-major: xt[p, q*4096 + k*512 + c] = xT[k*128+p, q*512+c]
        xt_e = np.ascontiguousarray(
            xT.reshape(KH, 128, NCH, 512).transpose(1, 2, 0, 3)
        ).reshape(128, KH * C).astype(ml_dtypes.bfloat16)
        # w1 lhsT pack: [m-tile][k-inner(p), k-tile*128 + m-inner]
        wpack = np.ascontiguousarray(
            gate_up_proj[e].reshape(KH, 128, 2 * MH, 128).transpose(2, 1, 0, 3)
        ).reshape(2 * MH, 128, KH * 128)
        w1_e = np.concatenate([wpack[:MH], wpack[MH:]], axis=-1).astype(
            ml_dtypes.bfloat16)  # [MH, 128, 2048] = gate|up
        w2_e = np.ascontiguousarray(
            down_proj[e].reshape(KI, 128, M2, 128).transpose(2, 1, 0, 3)
        ).reshape(M2, 128, KI * 128).astype(ml_dtypes.bfloat16)
        sc_e = np.broadcast_to(scale[e][None, :], (128, C)).astype(
            ml_dtypes.bfloat16).copy()
        return {"xt": xt_e, "w1": w1_e, "w2": w2_e, "sc": sc_e}

    from concurrent.futures import ThreadPoolExecutor
    with ThreadPoolExecutor(max_workers=E) as pool:
        in_maps = list(pool.map(_prep_expert, range(E)))
    return in_maps, perm_idx


def run_spmd(in_maps, **kwargs):
    global _CACHED_NC
    if _CACHED_NC is None:
        _CACHED_NC = _build_nc()
    return run_bass_kernel_spmd(
        _CACHED_NC, in_maps, core_ids=list(range(E)), **kwargs
    )


_CACHED_RUNNER = None


def _fast_run(in_maps):
    """Same semantics as run_bass_kernel_spmd under axon, but the jitted
    shard_map callable is built once and reused, avoiding per-call retrace."""
    global _CACHED_NC, _CACHED_RUNNER
    if _CACHED_RUNNER is None:
        if _CACHED_NC is None:
            _CACHED_NC = _build_nc()
        nc = _CACHED_NC
        import jax
        from jax.sharding import Mesh, PartitionSpec
        try:
            from jax.experimental.shard_map import shard_map
        except ImportError:
            from jax.shard_map import shard_map  # newer jax
        from concourse import bass2jax, mybir as _mybir
        bass2jax.install_neuronx_cc_hook()

        partition_name = (
            nc.partition_id_tensor.name if nc.partition_id_tensor else None
        )
        in_names, out_names, out_avals = [], [], []
        for alloc in nc.m.functions[0].allocations:
            if not isinstance(alloc, _mybir.MemoryLocationSet):
                continue
            name = alloc.memorylocations[0].name
            if alloc.kind == "ExternalInput":
                if name != partition_name:
                    in_names.append(name)
            elif alloc.kind == "ExternalOutput":
                out_names.append(name)
                out_avals.append(jax.core.ShapedArray(
                    tuple(alloc.tensor_shape), _mybir.dt.np(alloc.dtype)))
        n_params = len(in_names)
        n_outs = len(out_avals)
        all_in_names = list(in_names) + list(out_names)
        if partition_name is not None:
            all_in_names.append(partition_name)
        donate = tuple(range(n_params, n_params + n_outs))

        def _body(*args):
            operands = list(args)
            if partition_name is not None:
                operands.append(bass2jax.partition_id_tensor())
            outs = bass2jax._bass_exec_p.bind(
                *operands,
                out_avals=tuple(out_avals),
                in_names=tuple(all_in_names),
                out_names=tuple(out_names),
                lowering_input_output_aliases=(),
                sim_require_finite=True,
                sim_require_nnan=True,
                nc=nc,
            )
            return tuple(outs)

        devices = jax.devices()[:E]
        mesh = Mesh(np.array(devices), ("core",))
        in_specs = (PartitionSpec("core"),) * (n_params + n_outs)
        out_specs = (PartitionSpec("core"),) * n_outs
        sharded = jax.jit(
            shard_map(_body, mesh=mesh, in_specs=in_specs,
                      out_specs=out_specs, check_rep=False),
            donate_argnums=donate, keep_unused=True,
        )
        _CACHED_RUNNER = (sharded, in_names, out_names, out_avals)

    sharded, in_names, out_names, out_avals = _CACHED_RUNNER
    concat_in = [
        np.concatenate([np.asarray(m[name]) for m in in_maps], axis=0)
        for name in in_names
    ]
    concat_zeros = [
        np.zeros((E * a.shape[0], *a.shape[1:]), a.dtype) for a in out_avals
    ]
    out_arrs = sharded(*concat_in, *concat_zeros)
    results = [
        {name: np.asarray(out_arrs[i]).reshape(E, *out_avals[i].shape)[c]
         for i, name in enumerate(out_names)}
        for c in range(E)
    ]
    return results


def combine(res, perm_idx):
    # (E, C, H) of scaled expert outputs, then scatter-add back to tokens.
    results = res.results if hasattr(res, "results") else res
    y_flat = np.empty((E * C, H), dtype=np.float32)
    for e in range(E):
        yt_e = np.asarray(results[e]["yt"]).reshape(H, C).astype(np.float32)
        y_flat[e * C:(e + 1) * C] = yt_e.T
    out = np.zeros((T, H), dtype=np.float32)
    for k in range(K):
        idx = perm_idx[:, k]
        m = idx > 0
        out[m] += y_flat[idx[m] - 1]
    return out


def kernel(hidden_states, expert_affinities, expert_index, gate_up_proj, down_proj):
    in_maps, perm_idx = prepare_in_maps(
        hidden_states, expert_affinities, expert_index, gate_up_proj, down_proj
    )
    try:
        results = _fast_run(in_maps)
    except Exception:
        results = run_spmd(in_maps).results
    return combine(results, perm_idx)


# revision 3
# speedup vs baseline: 1.2012x; 1.2012x over previous
"""MoE expert-MLP kernel for Trainium2, expert-parallel across 8 NeuronCores.

Single pass over the full capacity C (no C-halving), all-bf16 operands,
minimal HBM traffic (~34MB per core vs 101MB for an f32r two-pass
variant). The PE streams back-to-back N=512 bf16 matmuls at the 216ns
floor (512 cycles @2.4GHz + NX overhead) with LDWEIGHTS fully hidden, so
the kernel runs at ~684us = 3072 MMs x 216ns + ~13us startup + ~11us
drain tail. (On hosts sitting in the P0 power state the PE clock is
2.0GHz and everything scales x1.2.)

Problem: T=8192 tokens, H=1024 hidden, I=4096 intermediate, E=8 experts,
top-K=2, capacity C = T*K/E = 2048 slots per expert. One expert per core.

Device program per core:
  phase 1: for each of 32 m-tile pairs of W1 (gate|up packed [128,2048]):
    gu^T = W1^T x^T accumulated over H in PSUM (pg, pu), evacuated as
    h^T = silu(g^T)*u^T into a resident bf16 ht buffer [I=4096, C].
  phase 2: y^T = W2^T h^T accumulated over I, scaled by the per-slot
    combine scale on evacuation, written out in bf16.

xt DRAM/SBUF layout is chunk-major [128, 4 chunks x (8 k-tiles x 512)] so
each 512-column chunk loads as one contiguous 1MB DMA.
"""

import sys

import numpy as np

try:
    import concourse.bass as bass  # noqa: F401
except ImportError:
    for p in ("/opt/trn_rl_repo", "/root/.axon_site/_ro/trn_rl_repo"):
        if p not in sys.path:
            sys.path.insert(0, p)

import ml_dtypes
import concourse.bass as bass
import concourse.tile as tile
from concourse import bacc, mybir
from concourse.bass_utils import run_bass_kernel_spmd

dt = mybir.dt

T, H, I, E, K = 8192, 1024, 4096, 8, 2
C = 2048  # min(T, ceil(T*K*1.0/E))
KH = H // 128    # 8  k-tiles for the first contraction
KI = I // 128    # 32 k-tiles for the second contraction
MH = I // 128    # 32 m-tile pairs (gate, up) in phase 1
M2 = H // 128    # 8  output tiles of y^T
NCH = C // 512   # 4  512-column chunks

_CACHED_NC = None


def _build_nc():
    nc = bacc.Bacc(None)
    xt = nc.dram_tensor("xt", [128, KH * C], dt.bfloat16, kind="ExternalInput")
    w1 = nc.dram_tensor("w1", [MH, 128, 2048], dt.bfloat16, kind="ExternalInput")
    w2 = nc.dram_tensor("w2", [M2, 128, KI * 128], dt.bfloat16, kind="ExternalInput")
    sc = nc.dram_tensor("sc", [128, C], dt.bfloat16, kind="ExternalInput")
    yt = nc.dram_tensor("yt", [M2, 128, C], dt.bfloat16, kind="ExternalOutput")

    with tile.TileContext(nc) as tc:
        with (
            tc.tile_pool(name="xt_p", bufs=1) as xt_p,
            tc.tile_pool(name="sc_p", bufs=1) as sc_p,
            tc.tile_pool(name="ht_p", bufs=1) as ht_p,
            tc.tile_pool(name="w1_p", bufs=2) as w1_p,
            tc.tile_pool(name="w2_p", bufs=2) as w2_p,
            tc.tile_pool(name="sg_p", bufs=4) as sg_p,
            tc.tile_pool(name="yb_p", bufs=2) as yb_p,
            tc.tile_pool(name="ps", bufs=2, space="PSUM") as ps,
            tc.tile_pool(name="ps_y", bufs=4, space="PSUM") as ps_y,
        ):
            xt_t = xt_p.tile([128, KH * C], dt.bfloat16, tag="xt", name="xt")
            sc_t = sc_p.tile([128, C], dt.bfloat16, tag="sc", name="sc")
            ht_t = [ht_p.tile([128, C], dt.bfloat16, tag=f"ht{i}", name=f"ht{i}")
                    for i in range(KI)]

            # Startup: xt chunk 0 gates the first MM group — split it
            # across both HWDGE issue engines (sync + scalar) so the two
            # halves issue concurrently; w1[0] likewise.
            w1t0 = w1_p.tile([128, 2048], dt.bfloat16, tag="w1", name="w1t0")
            half = KH * 256
            nc.sync.dma_start(xt_t[:, 0:half], xt[:, 0:half])
            nc.scalar.dma_start(xt_t[:, half:2 * half], xt[:, half:2 * half])
            nc.sync.dma_start(w1t0[:, 0:1024], w1[0][:, 0:1024])
            nc.scalar.dma_start(w1t0[:, 1024:2048], w1[0][:, 1024:2048])
            w1t1 = w1_p.tile([128, 2048], dt.bfloat16, tag="w1", name="w1t1")
            nc.scalar.dma_start(w1t1[:], w1[1])
            nc.sync.dma_start(
                xt_t[:, KH * 512:2 * KH * 512], xt[:, KH * 512:2 * KH * 512])
            for q in range(2, NCH):
                nc.sync.dma_start(
                    xt_t[:, q * KH * 512:(q + 1) * KH * 512],
                    xt[:, q * KH * 512:(q + 1) * KH * 512],
                )
            nc.sync.dma_start(sc_t[:], sc[:])

            # ---- phase 1: gu^T = W1^T x^T ; h^T = silu(g)*u (bf16) ----
            for mh in range(MH):
                if mh == 0:
                    w1t = w1t0
                elif mh == 1:
                    w1t = w1t1
                else:
                    w1t = w1_p.tile([128, 2048], dt.bfloat16, tag="w1",
                                    name="w1t")
                    nc.sync.dma_start(w1t[:], w1[mh])
                if mh == 16 or mh == 24:
                    # prefetch W2 for the phase transition
                    w2t = w2_p.tile([128, KI * 128], dt.bfloat16, tag="w2",
                                    name="w2pre")
                    nc.sync.dma_start(w2t[:], w2[0 if mh == 16 else 1])
                    if mh == 16:
                        w2_pre = [w2t]
                    else:
                        w2_pre.append(w2t)
                ht = ht_t[mh]
                for n in range(NCH):
                    base = n * KH * 512
                    pg = ps.tile([128, 512], dt.float32, tag="pg", name="pg")
                    pu = ps.tile([128, 512], dt.float32, tag="pu", name="pu")
                    for k in range(KH):
                        nc.tensor.matmul(
                            pg[:],
                            w1t[:, k * 128:(k + 1) * 128],
                            xt_t[:, base + k * 512:base + (k + 1) * 512],
                            start=(k == 0),
                            stop=(k == KH - 1),
                        )
                    for k in range(KH):
                        nc.tensor.matmul(
                            pu[:],
                            w1t[:, 1024 + k * 128:1024 + (k + 1) * 128],
                            xt_t[:, base + k * 512:base + (k + 1) * 512],
                            start=(k == 0),
                            stop=(k == KH - 1),
                        )
                    sg = sg_p.tile([128, 512], dt.float32, tag="sg", name="sg")
                    nc.scalar.activation(
                        sg[:], pg[:], mybir.ActivationFunctionType.Silu
                    )
                    nc.vector.tensor_mul(
                        ht[:, n * 512:(n + 1) * 512], sg[:], pu[:]
                    )

            # ---- phase 2: y^T = W2^T h^T, scaled on evacuation ----
            for m in range(M2):
                if m < 2:
                    w2t = w2_pre[m]
                else:
                    w2t = w2_p.tile([128, KI * 128], dt.bfloat16, tag="w2",
                                    name="w2t")
                    nc.sync.dma_start(w2t[:], w2[m])
                yb = yb_p.tile([128, C], dt.bfloat16, tag="yb", name="yb")
                for n in range(NCH):
                    py = ps_y.tile([128, 512], dt.float32, tag="py", name="py")
                    for k in range(KI):
                        nc.tensor.matmul(
                            py[:],
                            w2t[:, k * 128:(k + 1) * 128],
                            ht_t[k][:, n * 512:(n + 1) * 512],
                            start=(k == 0),
                            stop=(k == KI - 1),
                        )
                    nc.vector.tensor_mul(yb[:, n * 512:(n + 1) * 512], py[:],
                                         sc_t[:, n * 512:(n + 1) * 512])
                    if m == M2 - 1:
                        nc.sync.dma_start(yt[m][:, n * 512:(n + 1) * 512],
                                          yb[:, n * 512:(n + 1) * 512])
                if m < M2 - 1:
                    nc.sync.dma_start(yt[m], yb[:])
    nc.finalize()
    return nc


def _route(expert_affinities, expert_index):
    """Numpy port of the reference routing. Returns (tok_idx, valid, scale)."""
    mask = np.zeros((T, E), dtype=np.float32)
    rows = np.arange(T)[:, None]
    mask[rows, expert_index] = 1.0  # top-k entries are distinct per token
    position = np.cumsum(mask, axis=0, dtype=np.float32)  # 1-based
    mask = np.where(position > C, 0.0, mask)

    affin = np.where(mask == 0, 0.0, expert_affinities)
    denom = np.maximum(np.sum(np.abs(affin), axis=1, keepdims=True), 1e-12)
    affin = affin / denom

    offsets = np.arange(E, dtype=np.float32) * C
    pos_off = np.where(mask == 0, 0.0, position + offsets)
    perm_idx = np.take_along_axis(pos_off, expert_index, axis=1).astype(np.int32)

    tok_ids = np.broadcast_to(
        np.arange(1, T + 1, dtype=np.int32)[:, None], (T, K)
    )
    assignments = np.zeros(E * C + 1, dtype=np.int32)
    assignments[perm_idx.reshape(-1)] = tok_ids.reshape(-1)
    assignments = assignments[1:].reshape(E, C) - 1
    valid = assignments >= 0
    tok_idx = np.maximum(assignments, 0)

    scale = affin[tok_idx, np.arange(E)[:, None]] * valid.astype(np.float32)
    return tok_idx, valid, scale, perm_idx


def prepare_in_maps(hidden_states, expert_affinities, expert_index,
                    gate_up_proj, down_proj):
    hidden_states = np.asarray(hidden_states, dtype=np.float32)
    expert_affinities = np.asarray(expert_affinities, dtype=np.float32)
    expert_index = np.asarray(expert_index, dtype=np.int32)
    gate_up_proj = np.asarray(gate_up_proj, dtype=np.float32)
    down_proj = np.asarray(down_proj, dtype=np.float32)

    tok_idx, valid, scale, perm_idx = _route(expert_affinities, expert_index)

    def _prep_expert(e):
        x_e = hidden_states[tok_idx[e]]  # (C, H)
        xT = np.ascontiguousarray(x_e.T)  # (H, C)
        # chunk-major: xt[p, q*4096 + k*512 + c] = xT[k*128+p, q*512+c]
        xt_e = np.ascontiguousarray(
            xT.reshape(KH, 128, NCH, 512).transpose(1, 2, 0, 3)
        ).reshape(128, KH * C).astype(ml_dtypes.bfloat16)
        # w1 lhsT pack: [m-tile][k-inner(p), k-tile*128 + m-inner]
        wpack = np.ascontiguousarray(
            gate_up_proj[e].reshape(KH, 128, 2 * MH, 128).transpose(2, 1, 0, 3)
        ).reshape(2 * MH, 128, KH * 128)
        w1_e = np.concatenate([wpack[:MH], wpack[MH:]], axis=-1).astype(
            ml_dtypes.bfloat16)  # [MH, 128, 2048] = gate|up
        w2_e = np.ascontiguousarray(
            down_proj[e].reshape(KI, 128, M2, 128).transpose(2, 1, 0, 3)
        ).reshape(M2, 128, KI * 128).astype(ml_dtypes.bfloat16)
        sc_e = np.broadcast_to(scale[e][None, :], (128, C)).astype(
            ml_dtypes.bfloat16).copy()
        return {"xt": xt_e, "w1": w1_e, "w2": w2_e, "sc": sc_e}

    from concurrent.futures import ThreadPoolExecutor
    with ThreadPoolExecutor(max_workers=E) as pool:
        in_maps = list(pool.map(_prep_expert, range(E)))
    return in_maps, perm_idx


def run_spmd(in_maps, **kwargs):
    global _CACHED_NC
    if _CACHED_NC is None:
        _CACHED_NC = _build_nc()
    return run_bass_kernel_spmd(
        _CACHED_NC, in_maps, core_ids=list(range(E)), **kwargs
    )


_CACHED_RUNNER = None


def _fast_run(in_maps):
    """Same semantics as run_bass_kernel_spmd under axon, but the jitted
    shard_map callable is built once and reused, avoiding per-call retrace."""
    global _CACHED_NC, _CACHED_RUNNER
    if _CACHED_RUNNER is None:
        if _CACHED_NC is None:
            _CACHED_NC = _build_nc()
        nc = _CACHED_NC
        import jax
        from jax.sharding import Mesh, PartitionSpec
        try:
            from jax.experimental.shard_map import shard_map
        except ImportError:
            from jax.shard_map import shard_map  # newer jax
        from concourse import bass2jax, mybir as _mybir
        bass2jax.install_neuronx_cc_hook()

        partition_name = (
            nc.partition_id_tensor.name if nc.partition_id_tensor else None
        )
        in_names, out_names, out_avals = [], [], []
        for alloc in nc.m.functions[0].allocations:
            if not isinstance(alloc, _mybir.MemoryLocationSet):
                continue
            name = alloc.memorylocations[0].name
            if alloc.kind == "ExternalInput":
                if name != partition_name:
                    in_names.append(name)
            elif alloc.kind == "ExternalOutput":
                out_names.append(name)
                out_avals.append(jax.core.ShapedArray(
                    tuple(alloc.tensor_shape), _mybir.dt.np(alloc.dtype)))
        n_params = len(in_names)
        n_outs = len(out_avals)
        all_in_names = list(in_names) + list(out_names)
        if partition_name is not None:
            all_in_names.append(partition_name)
        donate = tuple(range(n_params, n_params + n_outs))

        def _body(*args):
            operands = list(args)
            if partition_name is not None:
                operands.append(bass2jax.partition_id_tensor())
            outs = bass2jax._bass_exec_p.bind(
                *operands,
                out_avals=tuple(out_avals),
                in_names=tuple(all_in_names),
                out_names=tuple(out_names),
                lowering_input_output_aliases=(),
                sim_require_finite=True,
                sim_require_nnan=True,
                nc=nc,
            )
            return tuple(outs)

        devices = jax.devices()[:E]
        mesh = Mesh(np.array(devices), ("core",))
        in_specs = (PartitionSpec("core"),) * (n_params + n_outs)
        out_specs = (PartitionSpec("core"),) * n_outs
        sharded = jax.jit(
            shard_map(_body, mesh=mesh, in_specs=in_specs,
                      out_specs=out_specs, check_rep=False),
            donate_argnums=donate, keep_unused=True,
        )
        _CACHED_RUNNER = (sharded, in_names, out_names, out_avals)

    sharded, in_names, out_names, out_avals = _CACHED_RUNNER
    concat_in = [
        np.concatenate([np.asarray(m[name]) for m in in_maps], axis=0)
        for name in in_names
    ]
    concat_zeros = [
        np.zeros((E * a.shape[0], *a.shape[1:]), a.dtype) for a in out_avals
    ]
    out_arrs = sharded(*concat_in, *concat_zeros)
    results = [
        {name: np.asarray(out_arrs[i]).reshape(E, *out_avals[i].shape)[c]
         for i, name in enumerate(out_names)}
        for c in range(E)
    ]
    return results


def combine(res, perm_idx):
    # (E, C, H) of scaled expert outputs, then scatter-add back to tokens.
    results = res.results if hasattr(res, "results") else res
    y_flat = np.empty((E * C, H), dtype=np.float32)
    for e in range(E):
        yt_e = np.asarray(results[e]["yt"]).reshape(H, C).astype(np.float32)
        y_flat[e * C:(e + 1) * C] = yt_e.T
    out = np.zeros((T, H), dtype=np.float32)
    for k in range(K):
        idx = perm_idx[:, k]
        m = idx > 0
        out[m] += y_flat[idx[m] - 1]
    return out


def kernel(hidden_states, expert_affinities, expert_index, gate_up_proj, down_proj):
    in_maps, perm_idx = prepare_in_maps(
        hidden_states, expert_affinities, expert_index, gate_up_proj, down_proj
    )
    try:
        results = _fast_run(in_maps)
    except Exception:
        results = run_spmd(in_maps).results
    return combine(results, perm_idx)


# revision 4
# speedup vs baseline: 1.2025x; 1.0011x over previous
"""MoE expert-MLP kernel for Trainium2, expert-parallel across 8 NeuronCores.

Single pass over the full capacity C (no C-halving), all-bf16 operands,
minimal HBM traffic (~34MB per core vs 101MB for an f32r two-pass
variant). The PE streams back-to-back N=512 bf16 matmuls at the 216ns
floor (512 cycles @2.4GHz + NX overhead) with LDWEIGHTS fully hidden and
a seamless phase-1 -> phase-2 transition, so the kernel runs at ~683us =
3072 MMs x 216ns + ~11us startup (HBM-ramp-ordered first loads) + ~11us
fixed drain tail. (On hosts sitting in the P0 power state the PE clock is
2.0GHz and everything scales x1.2.)

Problem: T=8192 tokens, H=1024 hidden, I=4096 intermediate, E=8 experts,
top-K=2, capacity C = T*K/E = 2048 slots per expert. One expert per core.

Device program per core:
  phase 1: for each of 32 m-tile pairs of W1 (gate|up packed [128,2048]):
    gu^T = W1^T x^T accumulated over H in PSUM (pg, pu), evacuated as
    h^T = silu(g^T)*u^T into a resident bf16 ht buffer [I=4096, C].
  phase 2: y^T = W2^T h^T accumulated over I, scaled by the per-slot
    combine scale on evacuation, written out in bf16.

xt DRAM/SBUF layout is chunk-major [128, 4 chunks x (8 k-tiles x 512)] so
each 512-column chunk loads as one contiguous 1MB DMA.
"""

import sys

import numpy as np

try:
    import concourse.bass as bass  # noqa: F401
except ImportError:
    for p in ("/opt/trn_rl_repo", "/root/.axon_site/_ro/trn_rl_repo"):
        if p not in sys.path:
            sys.path.insert(0, p)

import ml_dtypes
import concourse.bass as bass
import concourse.tile as tile
from concourse import bacc, mybir
from concourse.bass_utils import run_bass_kernel_spmd

dt = mybir.dt

T, H, I, E, K = 8192, 1024, 4096, 8, 2
C = 2048  # min(T, ceil(T*K*1.0/E))
KH = H // 128    # 8  k-tiles for the first contraction
KI = I // 128    # 32 k-tiles for the second contraction
MH = I // 128    # 32 m-tile pairs (gate, up) in phase 1
M2 = H // 128    # 8  output tiles of y^T
NCH = C // 512   # 4  512-column chunks

_CACHED_NC = None


def _build_nc():
    nc = bacc.Bacc(None)
    xt = nc.dram_tensor("xt", [128, KH * C], dt.bfloat16, kind="ExternalInput")
    w1 = nc.dram_tensor("w1", [MH, 128, 2048], dt.bfloat16, kind="ExternalInput")
    w2 = nc.dram_tensor("w2", [M2, 128, KI * 128], dt.bfloat16, kind="ExternalInput")
    sc = nc.dram_tensor("sc", [128, C], dt.bfloat16, kind="ExternalInput")
    yt = nc.dram_tensor("yt", [M2, 128, C], dt.bfloat16, kind="ExternalOutput")

    with tile.TileContext(nc) as tc:
        with (
            tc.tile_pool(name="xt_p", bufs=1) as xt_p,
            tc.tile_pool(name="sc_p", bufs=1) as sc_p,
            tc.tile_pool(name="ht_p", bufs=1) as ht_p,
            tc.tile_pool(name="w1_p", bufs=2) as w1_p,
            tc.tile_pool(name="w2_p", bufs=2) as w2_p,
            tc.tile_pool(name="sg_p", bufs=4) as sg_p,
            tc.tile_pool(name="yb_p", bufs=2) as yb_p,
            tc.tile_pool(name="ps", bufs=2, space="PSUM") as ps,
            tc.tile_pool(name="ps_y", bufs=4, space="PSUM") as ps_y,
        ):
            xt_t = xt_p.tile([128, KH * C], dt.bfloat16, tag="xt", name="xt")
            sc_t = sc_p.tile([128, C], dt.bfloat16, tag="sc", name="sc")
            ht_t = [ht_p.tile([128, C], dt.bfloat16, tag=f"ht{i}", name=f"ht{i}")
                    for i in range(KI)]

            # Startup: all 8 cores pull their first tiles simultaneously,
            # so HBM bandwidth (not issue rate) gates the ramp. Order by
            # need-time with the smallest critical piece (w1[0]) first, and
            # split chunk 0 into quarters so the k-loop starts on partial
            # data: quarter j carries k-slices {2j, 2j+1}.
            w1t0 = w1_p.tile([128, 2048], dt.bfloat16, tag="w1", name="w1t0")
            nc.sync.dma_start(w1t0[:], w1[0])
            for j in range(4):
                nc.sync.dma_start(xt_t[:, j * 1024:(j + 1) * 1024],
                                  xt[:, j * 1024:(j + 1) * 1024])
            nc.sync.dma_start(
                xt_t[:, KH * 512:2 * KH * 512], xt[:, KH * 512:2 * KH * 512])
            w1t1 = w1_p.tile([128, 2048], dt.bfloat16, tag="w1", name="w1t1")
            nc.sync.dma_start(w1t1[:], w1[1])
            for q in range(2, NCH):
                nc.sync.dma_start(
                    xt_t[:, q * KH * 512:(q + 1) * KH * 512],
                    xt[:, q * KH * 512:(q + 1) * KH * 512],
                )
            nc.sync.dma_start(sc_t[:], sc[:])

            # ---- phase 1: gu^T = W1^T x^T ; h^T = silu(g)*u (bf16) ----
            for mh in range(MH):
                if mh == 0:
                    w1t = w1t0
                elif mh == 1:
                    w1t = w1t1
                else:
                    w1t = w1_p.tile([128, 2048], dt.bfloat16, tag="w1",
                                    name="w1t")
                    nc.sync.dma_start(w1t[:], w1[mh])
                if mh == 16 or mh == 24:
                    # prefetch W2 for the phase transition
                    w2t = w2_p.tile([128, KI * 128], dt.bfloat16, tag="w2",
                                    name="w2pre")
                    nc.sync.dma_start(w2t[:], w2[0 if mh == 16 else 1])
                    if mh == 16:
                        w2_pre = [w2t]
                    else:
                        w2_pre.append(w2t)
                ht = ht_t[mh]
                for n in range(NCH):
                    base = n * KH * 512
                    pg = ps.tile([128, 512], dt.float32, tag="pg", name="pg")
                    pu = ps.tile([128, 512], dt.float32, tag="pu", name="pu")
                    for k in range(KH):
                        nc.tensor.matmul(
                            pg[:],
                            w1t[:, k * 128:(k + 1) * 128],
                            xt_t[:, base + k * 512:base + (k + 1) * 512],
                            start=(k == 0),
                            stop=(k == KH - 1),
                        )
                    for k in range(KH):
                        nc.tensor.matmul(
                            pu[:],
                            w1t[:, 1024 + k * 128:1024 + (k + 1) * 128],
                            xt_t[:, base + k * 512:base + (k + 1) * 512],
                            start=(k == 0),
                            stop=(k == KH - 1),
                        )
                    sg = sg_p.tile([128, 512], dt.float32, tag="sg", name="sg")
                    nc.scalar.activation(
                        sg[:], pg[:], mybir.ActivationFunctionType.Silu
                    )
                    nc.vector.tensor_mul(
                        ht[:, n * 512:(n + 1) * 512], sg[:], pu[:]
                    )

            # ---- phase 2: y^T = W2^T h^T, scaled on evacuation ----
            for m in range(M2):
                if m < 2:
                    w2t = w2_pre[m]
                else:
                    w2t = w2_p.tile([128, KI * 128], dt.bfloat16, tag="w2",
                                    name="w2t")
                    nc.sync.dma_start(w2t[:], w2[m])
                yb = yb_p.tile([128, C], dt.bfloat16, tag="yb", name="yb")
                for n in range(NCH):
                    py = ps_y.tile([128, 512], dt.float32, tag="py", name="py")
                    for k in range(KI):
                        nc.tensor.matmul(
                            py[:],
                            w2t[:, k * 128:(k + 1) * 128],
                            ht_t[k][:, n * 512:(n + 1) * 512],
                            start=(k == 0),
                            stop=(k == KI - 1),
                        )
                    nc.vector.tensor_mul(yb[:, n * 512:(n + 1) * 512], py[:],
                                         sc_t[:, n * 512:(n + 1) * 512])
                    if m == M2 - 1:
                        nc.sync.dma_start(yt[m][:, n * 512:(n + 1) * 512],
                                          yb[:, n * 512:(n + 1) * 512])
                if m < M2 - 1:
                    nc.sync.dma_start(yt[m], yb[:])
    nc.finalize()
    return nc


def _route(expert_affinities, expert_index):
    """Numpy port of the reference routing. Returns (tok_idx, valid, scale)."""
    mask = np.zeros((T, E), dtype=np.float32)
    rows = np.arange(T)[:, None]
    mask[rows, expert_index] = 1.0  # top-k entries are distinct per token
    position = np.cumsum(mask, axis=0, dtype=np.float32)  # 1-based
    mask = np.where(position > C, 0.0, mask)

    affin = np.where(mask == 0, 0.0, expert_affinities)
    denom = np.maximum(np.sum(np.abs(affin), axis=1, keepdims=True), 1e-12)
    affin = affin / denom

    offsets = np.arange(E, dtype=np.float32) * C
    pos_off = np.where(mask == 0, 0.0, position + offsets)
    perm_idx = np.take_along_axis(pos_off, expert_index, axis=1).astype(np.int32)

    tok_ids = np.broadcast_to(
        np.arange(1, T + 1, dtype=np.int32)[:, None], (T, K)
    )
    assignments = np.zeros(E * C + 1, dtype=np.int32)
    assignments[perm_idx.reshape(-1)] = tok_ids.reshape(-1)
    assignments = assignments[1:].reshape(E, C) - 1
    valid = assignments >= 0
    tok_idx = np.maximum(assignments, 0)

    scale = affin[tok_idx, np.arange(E)[:, None]] * valid.astype(np.float32)
    return tok_idx, valid, scale, perm_idx


def prepare_in_maps(hidden_states, expert_affinities, expert_index,
                    gate_up_proj, down_proj):
    hidden_states = np.asarray(hidden_states, dtype=np.float32)
    expert_affinities = np.asarray(expert_affinities, dtype=np.float32)
    expert_index = np.asarray(expert_index, dtype=np.int32)
    gate_up_proj = np.asarray(gate_up_proj, dtype=np.float32)
    down_proj = np.asarray(down_proj, dtype=np.float32)

    tok_idx, valid, scale, perm_idx = _route(expert_affinities, expert_index)

    def _prep_expert(e):
        x_e = hidden_states[tok_idx[e]]  # (C, H)
        xT = np.ascontiguousarray(x_e.T)  # (H, C)
        # chunk-major: xt[p, q*4096 + k*512 + c] = xT[k*128+p, q*512+c]
        xt_e = np.ascontiguousarray(
            xT.reshape(KH, 128, NCH, 512).transpose(1, 2, 0, 3)
        ).reshape(128, KH * C).astype(ml_dtypes.bfloat16)
        # w1 lhsT pack: [m-tile][k-inner(p), k-tile*128 + m-inner]
        wpack = np.ascontiguousarray(
            gate_up_proj[e].reshape(KH, 128, 2 * MH, 128).transpose(2, 1, 0, 3)
        ).reshape(2 * MH, 128, KH * 128)
        w1_e = np.concatenate([wpack[:MH], wpack[MH:]], axis=-1).astype(
            ml_dtypes.bfloat16)  # [MH, 128, 2048] = gate|up
        w2_e = np.ascontiguousarray(
            down_proj[e].reshape(KI, 128, M2, 128).transpose(2, 1, 0, 3)
        ).reshape(M2, 128, KI * 128).astype(ml_dtypes.bfloat16)
        sc_e = np.broadcast_to(scale[e][None, :], (128, C)).astype(
            ml_dtypes.bfloat16).copy()
        return {"xt": xt_e, "w1": w1_e, "w2": w2_e, "sc": sc_e}

    from concurrent.futures import ThreadPoolExecutor
    with ThreadPoolExecutor(max_workers=E) as pool:
        in_maps = list(pool.map(_prep_expert, range(E)))
    return in_maps, perm_idx


def run_spmd(in_maps, **kwargs):
    global _CACHED_NC
    if _CACHED_NC is None:
        _CACHED_NC = _build_nc()
    return run_bass_kernel_spmd(
        _CACHED_NC, in_maps, core_ids=list(range(E)), **kwargs
    )


_CACHED_RUNNER = None


def _fast_run(in_maps):
    """Same semantics as run_bass_kernel_spmd under axon, but the jitted
    shard_map callable is built once and reused, avoiding per-call retrace."""
    global _CACHED_NC, _CACHED_RUNNER
    if _CACHED_RUNNER is None:
        if _CACHED_NC is None:
            _CACHED_NC = _build_nc()
        nc = _CACHED_NC
        import jax
        from jax.sharding import Mesh, PartitionSpec
        try:
            from jax.experimental.shard_map import shard_map
        except ImportError:
            from jax.shard_map import shard_map  # newer jax
        from concourse import bass2jax, mybir as _mybir
        bass2jax.install_neuronx_cc_hook()

        partition_name = (
            nc.partition_id_tensor.name if nc.partition_id_tensor else None
        )
        in_names, out_names, out_avals = [], [], []
        for alloc in nc.m.functions[0].allocations:
            if not isinstance(alloc, _mybir.MemoryLocationSet):
                continue
            name = alloc.memorylocations[0].name
            if alloc.kind == "ExternalInput":
                if name != partition_name:
                    in_names.append(name)
            elif alloc.kind == "ExternalOutput":
                out_names.append(name)
                out_avals.append(jax.core.ShapedArray(
                    tuple(alloc.tensor_shape), _mybir.dt.np(alloc.dtype)))
        n_params = len(in_names)
        n_outs = len(out_avals)
        all_in_names = list(in_names) + list(out_names)
        if partition_name is not None:
            all_in_names.append(partition_name)
        donate = tuple(range(n_params, n_params + n_outs))

        def _body(*args):
            operands = list(args)
            if partition_name is not None:
                operands.append(bass2jax.partition_id_tensor())
            outs = bass2jax._bass_exec_p.bind(
                *operands,
                out_avals=tuple(out_avals),
                in_names=tuple(all_in_names),
                out_names=tuple(out_names),
                lowering_input_output_aliases=(),
                sim_require_finite=True,
                sim_require_nnan=True,
                nc=nc,
            )
            return tuple(outs)

        devices = jax.devices()[:E]
        mesh = Mesh(np.array(devices), ("core",))
        in_specs = (PartitionSpec("core"),) * (n_params + n_outs)
        out_specs = (PartitionSpec("core"),) * n_outs
        sharded = jax.jit(
            shard_map(_body, mesh=mesh, in_specs=in_specs,
                      out_specs=out_specs, check_rep=False),
            donate_argnums=donate, keep_unused=True,
        )
        _CACHED_RUNNER = (sharded, in_names, out_names, out_avals)

    sharded, in_names, out_names, out_avals = _CACHED_RUNNER
    concat_in = [
        np.concatenate([np.asarray(m[name]) for m in in_maps], axis=0)
        for name in in_names
    ]
    concat_zeros = [
        np.zeros((E * a.shape[0], *a.shape[1:]), a.dtype) for a in out_avals
    ]
    out_arrs = sharded(*concat_in, *concat_zeros)
    results = [
        {name: np.asarray(out_arrs[i]).reshape(E, *out_avals[i].shape)[c]
         for i, name in enumerate(out_names)}
        for c in range(E)
    ]
    return results


def combine(res, perm_idx):
    # (E, C, H) of scaled expert outputs, then scatter-add back to tokens.
    results = res.results if hasattr(res, "results") else res
    y_flat = np.empty((E * C, H), dtype=np.float32)
    for e in range(E):
        yt_e = np.asarray(results[e]["yt"]).reshape(H, C).astype(np.float32)
        y_flat[e * C:(e + 1) * C] = yt_e.T
    out = np.zeros((T, H), dtype=np.float32)
    for k in range(K):
        idx = perm_idx[:, k]
        m = idx > 0
        out[m] += y_flat[idx[m] - 1]
    return out


def kernel(hidden_states, expert_affinities, expert_index, gate_up_proj, down_proj):
    in_maps, perm_idx = prepare_in_maps(
        hidden_states, expert_affinities, expert_index, gate_up_proj, down_proj
    )
    try:
        results = _fast_run(in_maps)
    except Exception:
        results = run_spmd(in_maps).results
    return combine(results, perm_idx)
